# revision 5
# baseline (speedup 1.0000x reference)
"""Multi-head attention (B=4, N=2048, DIM=1024, H=16, HD=64) on 8 TRN2 cores.

Sharding: tensor-parallel over heads - 2 heads per core. The reference omits
the output projection, so each core's output is a disjoint 128-column slice of
the final [B, N, 1024]; no collectives are needed.

Per-core device kernel (bf16 compute, fp32 PSUM accumulation):
  - QKV projection from a single pass over x^T: q^T,k^T produced transposed
    [outch, tokens] (weights stationary), v produced natural [tokens, outch]
    (x tiles stationary).
  - scores^T = k^T.T @ q^T per (batch, head): K=64 contraction; head A lives
    on partitions 0-63 and head B on 64-127 (row-tiled, concurrent).
  - exp on ScalarE over two-bank [128, 1024] PSUM tiles -> bf16 SBUF pair
    tiles [128, 2048].
  - out^T = [vA|vB]^T @ expT: both heads packed in the column dim (M=128),
    two concurrent col-tiled matmuls per k-tile (tile_position (0,0)/(0,64)).
  - softmax denominator: bf16 pairwise add-tree over the exp tiles (DVE),
    then two ones[128,64] matmuls produce den replicated across partitions
    directly in the bcs layout; DVE fast-reciprocal + multiply + bias.
"""

import numpy as np
import ml_dtypes

import concourse.bacc as bacc
import concourse.mybir as mybir
from concourse.bass_utils import run_bass_kernel_spmd
from concourse.tile import TileContext

B, N, DIM, H = 4, 2048, 1024, 16
HD = DIM // H
SCALE = 1.0 / np.sqrt(HD)
TOK = B * N               # 8192 tokens
NCORES = 8
HPC = H // NCORES         # heads per core = 2

BF16 = mybir.dt.bfloat16
F32 = mybir.dt.float32
AF = mybir.ActivationFunctionType

NT = TOK // 512           # 16 token tiles of 512 for the projection
KT = 8                    # 1024 / 128 contraction tiles
QT = N // 512             # 4 q tiles per (b, h)
KTOK = N // 128           # 16 k-token tiles per (b, h)

# per-wave kt indices whose exp is computed on DVE (Schraudolph) instead of
# ScalarE, to balance the two engines. Empty for step 1.
SCHRAUD_KT = ()
SCH_A = 128.0 * np.log2(np.e)
SCH_B = 16256.0 - 7.4


def build_graph():
    nc = bacc.Bacc("TRN2", target_bir_lowering=False, debug=False)
    xt = nc.declare_dram_parameter("xt", [DIM, TOK], BF16, isOutput=False)
    wqk = nc.declare_dram_parameter("wqk", [DIM, 2 * HPC * HD], BF16, isOutput=False)
    wv = nc.declare_dram_parameter("wv", [DIM, HPC * HD], BF16, isOutput=False)
    bqk = nc.declare_dram_parameter("bqk", [2 * HPC * HD, 1], F32, isOutput=False)
    bvq = nc.declare_dram_parameter("bvq", [HPC * HD, 1], F32, isOutput=False)
    out = nc.declare_dram_parameter("out", [B, HPC * HD, N], F32, isOutput=True)
    NTB = N // 512            # 4 proj token-tiles per batch
    KTOK_B = N // 128         # 16 k-token tiles per batch

    with TileContext(nc) as tc:
        with (
            tc.tile_pool(name="const", bufs=1) as constp,
            tc.tile_pool(name="qk", bufs=1) as qkp,
            tc.tile_pool(name="xin", bufs=4) as xinp,
            tc.tile_pool(name="exps", bufs=18) as expp,
            tc.tile_pool(name="tree", bufs=2) as treep,
            tc.tile_pool(name="esum", bufs=2) as esump,
            tc.tile_pool(name="bcs", bufs=2) as bcsp,
            tc.tile_pool(name="outs", bufs=2) as outp,
        ):
            # ---- constants ----
            wqk_s = constp.tile([128, KT * 256], BF16)
            nc.sync.dma_start(
                out=wqk_s.rearrange("p (kt j) -> p kt j", kt=KT),
                in_=wqk.rearrange("(kt p) j -> p kt j", p=128))
            wv_s = constp.tile([128, KT * 128], BF16)
            nc.sync.dma_start(
                out=wv_s.rearrange("p (kt j) -> p kt j", kt=KT),
                in_=wv.rearrange("(kt p) j -> p kt j", p=128))
            bqk_s = constp.tile([128, 2], F32)
            for mt in range(2):
                nc.sync.dma_start(out=bqk_s[:, mt:mt + 1],
                                  in_=bqk[mt * 128:(mt + 1) * 128, :])
            bvq_s = constp.tile([128, 1], F32)
            nc.sync.dma_start(out=bvq_s[:, :], in_=bvq[:, :])
            ones = constp.tile([128, HD], BF16)
            nc.vector.memset(ones[:, :], 1.0)

            # per-batch activation tensors
            q_sb = [qkp.tile([128, N], BF16, name=f"q_sb{_b}") for _b in range(B)]
            k_sb = [qkp.tile([128, N], BF16, name=f"k_sb{_b}") for _b in range(B)]
            v_sb = [qkp.tile([128, KTOK_B * 128], BF16, name=f"v_sb{_b}")
                    for _b in range(B)]

            with (
                tc.tile_pool(name="qkps", bufs=1, space="PSUM") as qkps,
                tc.tile_pool(name="vps", bufs=1, space="PSUM") as vps,
                tc.tile_pool(name="sps", bufs=2, space="PSUM") as sps,
                tc.tile_pool(name="avps", bufs=1, space="PSUM") as avps,
                tc.tile_pool(name="denps", bufs=1, space="PSUM") as denps,
            ):
                xnt_tiles = {}

                def emit_group(seg):
                    kind = seg[0]
                    if kind == "load":
                        nt = seg[1]
                        xnt = xinp.tile([128, KT * 512], BF16, name="xnt")
                        nc.sync.dma_start(
                            out=xnt.rearrange("p (kt j) -> p kt j", kt=KT),
                            in_=xt.rearrange("(kt p) tok -> p kt tok", p=128)[
                                :, :, nt * 512:(nt + 1) * 512])
                        xnt_tiles[nt] = xnt
                    elif kind == "qk":
                        _, nt, mt = seg
                        bb, ntb = nt // NTB, nt % NTB
                        xnt = xnt_tiles[nt]
                        ps = qkps.tile([128, 512], F32, name="ps", tag="ps")
                        for kt in range(KT):
                            nc.tensor.matmul(
                                ps[:, :],
                                lhsT=wqk_s[:, kt * 256 + mt * 128: kt * 256 + (mt + 1) * 128],
                                rhs=xnt[:, kt * 512:(kt + 1) * 512],
                                start=(kt == 0), stop=(kt == KT - 1))
                        dst = q_sb[bb] if mt == 0 else k_sb[bb]
                        nc.vector.tensor_scalar_add(
                            dst[:, ntb * 512:(ntb + 1) * 512], ps[:, :],
                            bqk_s[:, mt:mt + 1])
                    elif kind == "v":
                        _, nt, sub = seg
                        bb, ntb = nt // NTB, nt % NTB
                        xnt = xnt_tiles[nt]
                        ttb = ntb * 4 + sub
                        vp = vps.tile([128, 128], F32, name="vp", tag="vp")
                        for kt in range(KT):
                            nc.tensor.matmul(
                                vp[:, :],
                                lhsT=xnt[:, kt * 512 + sub * 128: kt * 512 + (sub + 1) * 128],
                                rhs=wv_s[:, kt * 128:(kt + 1) * 128],
                                start=(kt == 0), stop=(kt == KT - 1))
                        nc.vector.tensor_copy(
                            v_sb[bb][:, ttb * 128:(ttb + 1) * 128], vp[:, :])

                GCOST = {"load": 0.1, "qk": 2.2, "v": 0.75}

                def proj_groups(bb):
                    segs = []
                    for ntb in range(NTB):
                        nt = bb * NTB + ntb
                        segs.append(("load", nt))
                        for mt in range(2):
                            segs.append(("qk", nt, mt))
                        for sub in range(4):
                            segs.append(("v", nt, sub))
                    return segs

                from collections import deque
                filler = deque()

                def emit_av(b_, pairs_, kt):
                    ep = pairs_[kt // 2]
                    off = (kt % 2) * 1024
                    pav_ = pav[0]
                    for h in range(2):
                        nc.tensor.matmul(
                            pav_[h * 64:(h + 1) * 64, :],
                            lhsT=v_sb[b_][:, kt * 128 + h * 64: kt * 128 + (h + 1) * 64],
                            rhs=ep[:, off + h * 512: off + (h + 1) * 512],
                            start=(kt == 0), stop=(kt == KTOK_B - 1),
                            tile_position=(0, h * 64),
                            skip_group_check=True)

                def emit_tail(b_, qt_, esum_):
                    # denominator: ones[128,64] matmuls -> den replicated
                    # across partitions, directly in the bcs layout
                    dps = denps.tile([128, 512], F32, name="dn", tag="dn")
                    for h in range(2):
                        nc.tensor.matmul(
                            dps[h * 64:(h + 1) * 64, :],
                            lhsT=ones[:, :],
                            rhs=esum_[:, h * 512:(h + 1) * 512],
                            start=True, stop=True,
                            tile_position=(0, h * 64),
                            skip_group_check=True)
                    rc = bcsp.tile([128, 512], F32, name="rc", tag="rc")
                    nc.vector.reciprocal_approx_fast(rc[:, :], dps[:, :])
                    ot = outp.tile([128, 512], F32, name="ot", tag="ot")
                    nc.vector.tensor_mul(ot[:, :], pav[0][:, :], rc[:, :])
                    ot2 = outp.tile([128, 512], F32, name="ot2", tag="ot2")
                    nc.gpsimd.tensor_scalar_add(ot2[:, :], ot[:, :],
                                                bvq_s[:, 0:1])
                    nc.sync.dma_start(
                        out=out[b_, :, qt_ * 512:(qt_ + 1) * 512],
                        in_=ot2[:, :])

                # batch-0 projection: first token-tile up front (enough for
                # wave 0's q and the first 4 k-tiles), rest via the filler
                segs0 = proj_groups(0)
                for seg in segs0[:3]:            # load0, qk(0,0), qk(0,1)
                    emit_group(seg)
                rest0 = segs0[3:]
                # reorder: remaining loads+qk first (k-data feeds the score
                # stream), v segments after
                filler.extend([s for s in rest0 if s[0] != "v"])
                filler.extend([s for s in rest0 if s[0] == "v"])

                prev = None        # (b, qt, pairs, esum) of previous wave
                pav = [None]
                for b in range(B):
                    for qt in range(QT):
                        if qt == 0 and b + 1 < B:
                            filler.extend(proj_groups(b + 1))
                        qcol = qt * 512
                        if prev is not None:
                            pav[0] = avps.tile([128, 512], F32, name="av",
                                               tag="av", bufs=1)
                        pairs = []
                        lvlA = []
                        lvlB = []
                        budget_per_kt = 1.3 if b == 0 else 0.85
                        for kt in range(KTOK_B):
                            kcol = kt * 128
                            half = kt % 2
                            if half == 0:
                                ep = expp.tile([128, 2048], BF16, name="e2",
                                               tag="e2")
                                pairs.append(ep)
                            s2 = sps.tile([128, 1024], F32, name="s2", tag="s2")
                            for h in range(2):
                                nc.tensor.matmul(
                                    s2[:, h * 512:(h + 1) * 512],
                                    lhsT=k_sb[b][h * 64:(h + 1) * 64, kcol:kcol + 128],
                                    rhs=q_sb[b][h * 64:(h + 1) * 64, qcol:qcol + 512],
                                    start=True, stop=True,
                                    tile_position=(h * 64, 0))
                            dst = pairs[-1][:, half * 1024:(half + 1) * 1024]
                            if kt in SCHRAUD_KT:
                                nc.vector.tensor_scalar(
                                    dst.bitcast(mybir.dt.int16), s2[:, :],
                                    SCH_A, SCH_B,
                                    mybir.AluOpType.mult, mybir.AluOpType.add)
                            else:
                                nc.scalar.activation(dst, s2[:, :], AF.Exp)
                            # previous wave's AV rides this wave's kt slots
                            if prev is not None:
                                emit_av(prev[0], prev[2], kt)
                            # denominator add-tree (bf16, pairwise)
                            if kt % 4 == 3:
                                a = treep.tile([128, 2048], BF16, name="tA",
                                               tag="tA")
                                nc.vector.tensor_add(a[:, :], pairs[-2][:, :],
                                                     pairs[-1][:, :])
                                lvlA.append(a)
                                if len(lvlA) % 2 == 0:
                                    bt = treep.tile([128, 2048], BF16,
                                                    name="tB", tag="tB")
                                    nc.vector.tensor_add(bt[:, :],
                                                         lvlA[-2][:, :],
                                                         lvlA[-1][:, :])
                                    lvlB.append(bt)
                            budget = budget_per_kt
                            while filler and budget > 0:
                                seg = filler.popleft()
                                budget -= GCOST[seg[0]]
                                emit_group(seg)
                        ct = treep.tile([128, 2048], BF16, name="tC", tag="tC")
                        nc.vector.tensor_add(ct[:, :], lvlB[0][:, :],
                                             lvlB[1][:, :])
                        esum = esump.tile([128, 1024], BF16, name="es",
                                          tag="es")
                        nc.vector.tensor_add(esum[:, :], ct[:, 0:1024],
                                             ct[:, 1024:2048])
                        if prev is not None:
                            emit_tail(prev[0], prev[1], prev[3])
                        prev = (b, qt, pairs, esum)
                # drain: last wave's AV + tail
                while filler:
                    emit_group(filler.popleft())
                pav[0] = avps.tile([128, 512], F32, name="av", tag="av",
                                   bufs=1)
                for kt in range(KTOK_B):
                    emit_av(prev[0], prev[2], kt)
                emit_tail(prev[0], prev[1], prev[3])
    nc.compile()
    return nc


_GRAPH = None


def _get_graph():
    global _GRAPH
    if _GRAPH is None:
        _GRAPH = build_graph()
    return _GRAPH


def _make_in_maps(x, w_qkv, b_qkv):
    bf = ml_dtypes.bfloat16
    xt = np.ascontiguousarray(x.reshape(TOK, DIM).T).astype(bf)
    in_maps = []
    for c in range(NCORES):
        hA, hB = HPC * c, HPC * c + 1
        rq = [w_qkv[h * HD:(h + 1) * HD] * SCALE for h in (hA, hB)]
        rk = [w_qkv[DIM + h * HD: DIM + (h + 1) * HD] for h in (hA, hB)]
        rv = [w_qkv[2 * DIM + h * HD: 2 * DIM + (h + 1) * HD] for h in (hA, hB)]
        wqk_c = np.ascontiguousarray(np.concatenate(rq + rk, axis=0).T).astype(bf)
        wv_c = np.ascontiguousarray(np.concatenate(rv, axis=0).T).astype(bf)
        bq = [b_qkv[h * HD:(h + 1) * HD] * SCALE for h in (hA, hB)]
        bk = [b_qkv[DIM + h * HD: DIM + (h + 1) * HD] for h in (hA, hB)]
        bvc = [b_qkv[2 * DIM + h * HD: 2 * DIM + (h + 1) * HD] for h in (hA, hB)]
        bqk_c = np.concatenate(bq + bk).astype(np.float32).reshape(-1, 1)
        bvq_c = np.concatenate(bvc).astype(np.float32).reshape(-1, 1)
        in_maps.append({"xt": xt, "wqk": wqk_c, "wv": wv_c,
                        "bqk": np.ascontiguousarray(bqk_c),
                        "bvq": np.ascontiguousarray(bvq_c)})
    return in_maps


def _run(x, w_qkv, b_qkv, trace=False, tmpdir=None):
    nc = _get_graph()
    in_maps = _make_in_maps(np.asarray(x, dtype=np.float32),
                            np.asarray(w_qkv, dtype=np.float32),
                            np.asarray(b_qkv, dtype=np.float32))
    res = run_bass_kernel_spmd(nc, in_maps, core_ids=list(range(NCORES)),
                               trace=trace, tmpdir=tmpdir)
    full = np.empty((B, N, DIM), dtype=np.float32)
    for c in range(NCORES):
        oc = res.results[c]["out"]          # [B, 128, N]
        full[:, :, c * HPC * HD:(c + 1) * HPC * HD] = oc.transpose(0, 2, 1)
    return full, res


def kernel(x, w_qkv, b_qkv):
    full, _ = _run(x, w_qkv, b_qkv, trace=False)
    return full


# revision 11
# speedup vs baseline: 1.3286x; 1.3286x over previous
"""Multi-head attention (B=4, N=2048, DIM=1024, H=16, HD=64) on 8 TRN2 cores.

Sharding: tensor-parallel over heads - 2 heads per core. The reference omits
the output projection, so each core's output is a disjoint 128-column slice of
the final [B, N, 1024]; no collectives are needed.

Per-core device kernel (bf16 compute, fp32 PSUM accumulation):
  - QKV projection from a single pass over x^T: q^T,k^T produced transposed
    [outch, tokens] (weights stationary), v produced natural [tokens, outch]
    (x tiles stationary).
  - scores^T = k^T.T @ q^T per (batch, head): K=64 contraction; head A lives
    on partitions 0-63 and head B on 64-127 (row-tiled, concurrent).
  - exp on ScalarE over two-bank [128, 1024] PSUM tiles -> bf16 SBUF pair
    tiles [128, 2048].
  - out^T = [vA|vB]^T @ expT: both heads packed in the column dim (M=128),
    two concurrent col-tiled matmuls per k-tile (tile_position (0,0)/(0,64)).
  - softmax denominator: bf16 pairwise add-tree over the exp tiles (DVE),
    then two ones[128,64] matmuls produce den replicated across partitions
    directly in the bcs layout; DVE fast-reciprocal + multiply + bias.
"""

import numpy as np
import ml_dtypes

import concourse.bacc as bacc
import concourse.mybir as mybir
from concourse.bass_utils import run_bass_kernel_spmd
from concourse.tile import TileContext

B, N, DIM, H = 4, 2048, 1024, 16
HD = DIM // H
SCALE = 1.0 / np.sqrt(HD)
TOK = B * N               # 8192 tokens
NCORES = 8
HPC = H // NCORES         # heads per core = 2

BF16 = mybir.dt.bfloat16
F32 = mybir.dt.float32
AF = mybir.ActivationFunctionType

NT = TOK // 512           # 16 token tiles of 512 for the projection
KT = 8                    # 1024 / 128 contraction tiles
QT = N // 512             # 4 q tiles per (b, h)
KTOK = N // 128           # 16 k-token tiles per (b, h)

# per-wave kt indices whose exp is computed on DVE (Schraudolph) instead of
# ScalarE, to balance the two engines. Empty for step 1.
SCHRAUD_KT = ()
SCH_A = 128.0 * np.log2(np.e)
SCH_B = 16256.0 - 7.4


def build_graph():
    nc = bacc.Bacc("TRN2", target_bir_lowering=False, debug=False)
    xt = nc.declare_dram_parameter("xt", [DIM, TOK], BF16, isOutput=False)
    wqk = nc.declare_dram_parameter("wqk", [DIM, 2 * HPC * HD], BF16, isOutput=False)
    wv = nc.declare_dram_parameter("wv", [DIM, HPC * HD], BF16, isOutput=False)
    bqk = nc.declare_dram_parameter("bqk", [2 * HPC * HD, 1], F32, isOutput=False)
    bvq = nc.declare_dram_parameter("bvq", [HPC * HD, 1], F32, isOutput=False)
    out = nc.declare_dram_parameter("out", [B, HPC * HD, N], F32, isOutput=True)
    NTB = N // 512            # 4 proj token-tiles per batch
    KTOK_B = N // 128         # 16 k-token tiles per batch

    with TileContext(nc) as tc:
        with (
            tc.tile_pool(name="const", bufs=1) as constp,
            tc.tile_pool(name="qk", bufs=1) as qkp,
            tc.tile_pool(name="xin", bufs=4) as xinp,
            tc.tile_pool(name="exps", bufs=18) as expp,
            tc.tile_pool(name="tree", bufs=2) as treep,
            tc.tile_pool(name="esum", bufs=2) as esump,
            tc.tile_pool(name="bcs", bufs=2) as bcsp,
            tc.tile_pool(name="outs", bufs=2) as outp,
        ):
            # ---- constants (spread across engine DMA queues so the first
            # x-tile load isn't stuck behind them on the sync queue) ----
            wqk_s = constp.tile([128, KT * 256], BF16)
            nc.scalar.dma_start(
                out=wqk_s.rearrange("p (kt j) -> p kt j", kt=KT),
                in_=wqk.rearrange("(kt p) j -> p kt j", p=128))
            wv_s = constp.tile([128, KT * 128], BF16)
            nc.gpsimd.dma_start(
                out=wv_s.rearrange("p (kt j) -> p kt j", kt=KT),
                in_=wv.rearrange("(kt p) j -> p kt j", p=128))
            bqk_s = constp.tile([128, 2], F32)
            for mt in range(2):
                nc.gpsimd.dma_start(out=bqk_s[:, mt:mt + 1],
                                    in_=bqk[mt * 128:(mt + 1) * 128, :])
            bvq_s = constp.tile([128, 1], F32)
            nc.gpsimd.dma_start(out=bvq_s[:, :], in_=bvq[:, :])
            ones = constp.tile([128, HD], BF16)
            nc.vector.memset(ones[:, :], 1.0)

            # per-batch activation tensors
            q_sb = [qkp.tile([128, N], BF16, name=f"q_sb{_b}") for _b in range(B)]
            k_sb = [qkp.tile([128, N], BF16, name=f"k_sb{_b}") for _b in range(B)]
            v_sb = [qkp.tile([128, KTOK_B * 128], BF16, name=f"v_sb{_b}")
                    for _b in range(B)]

            with (
                tc.tile_pool(name="qkps", bufs=1, space="PSUM") as qkps,
                tc.tile_pool(name="vps", bufs=1, space="PSUM") as vps,
                tc.tile_pool(name="sps", bufs=2, space="PSUM") as sps,
                tc.tile_pool(name="avps", bufs=1, space="PSUM") as avps,
                tc.tile_pool(name="denps", bufs=1, space="PSUM") as denps,
            ):
                xnt_tiles = {}

                def emit_group(seg):
                    kind = seg[0]
                    if kind == "load":
                        nt = seg[1]
                        xnt = xinp.tile([128, KT * 512], BF16, name="xnt")
                        nc.sync.dma_start(
                            out=xnt.rearrange("p (kt j) -> p kt j", kt=KT),
                            in_=xt.rearrange("(kt p) tok -> p kt tok", p=128)[
                                :, :, nt * 512:(nt + 1) * 512])
                        xnt_tiles[nt] = xnt
                    elif kind == "qk":
                        _, nt, mt = seg
                        bb, ntb = nt // NTB, nt % NTB
                        xnt = xnt_tiles[nt]
                        ps = qkps.tile([128, 512], F32, name="ps", tag="ps")
                        for kt in range(KT):
                            nc.tensor.matmul(
                                ps[:, :],
                                lhsT=wqk_s[:, kt * 256 + mt * 128: kt * 256 + (mt + 1) * 128],
                                rhs=xnt[:, kt * 512:(kt + 1) * 512],
                                start=(kt == 0), stop=(kt == KT - 1))
                        dst = q_sb[bb] if mt == 0 else k_sb[bb]
                        nc.vector.tensor_scalar_add(
                            dst[:, ntb * 512:(ntb + 1) * 512], ps[:, :],
                            bqk_s[:, mt:mt + 1])
                    elif kind == "v":
                        _, nt, sub = seg
                        bb, ntb = nt // NTB, nt % NTB
                        xnt = xnt_tiles[nt]
                        ttb = ntb * 4 + sub
                        vp = vps.tile([128, 128], F32, name="vp", tag="vp")
                        for kt in range(KT):
                            nc.tensor.matmul(
                                vp[:, :],
                                lhsT=xnt[:, kt * 512 + sub * 128: kt * 512 + (sub + 1) * 128],
                                rhs=wv_s[:, kt * 128:(kt + 1) * 128],
                                start=(kt == 0), stop=(kt == KT - 1))
                        nc.vector.tensor_copy(
                            v_sb[bb][:, ttb * 128:(ttb + 1) * 128], vp[:, :])

                GCOST = {"load": 0.1, "qk": 2.2, "v": 0.75}

                def proj_groups(bb):
                    segs = []
                    for ntb in range(NTB):
                        nt = bb * NTB + ntb
                        segs.append(("load", nt))
                        for mt in range(2):
                            segs.append(("qk", nt, mt))
                        for sub in range(4):
                            segs.append(("v", nt, sub))
                    return segs

                from collections import deque
                filler = deque()

                def emit_av(b_, pairs_, kt):
                    ep = pairs_[kt // 2]
                    off = (kt % 2) * 1024
                    pav_ = pav[0]
                    for h in range(2):
                        nc.tensor.matmul(
                            pav_[h * 64:(h + 1) * 64, :],
                            lhsT=v_sb[b_][:, kt * 128 + h * 64: kt * 128 + (h + 1) * 64],
                            rhs=ep[:, off + h * 512: off + (h + 1) * 512],
                            start=(kt == 0), stop=(kt == KTOK_B - 1),
                            tile_position=(0, h * 64),
                            skip_group_check=True)

                def emit_tail(b_, qt_, esum_):
                    # denominator: ones[128,64] matmuls -> den replicated
                    # across partitions, directly in the bcs layout
                    dps = denps.tile([128, 512], F32, name="dn", tag="dn")
                    for h in range(2):
                        nc.tensor.matmul(
                            dps[h * 64:(h + 1) * 64, :],
                            lhsT=ones[:, :],
                            rhs=esum_[:, h * 512:(h + 1) * 512],
                            start=True, stop=True,
                            tile_position=(0, h * 64),
                            skip_group_check=True)
                    rc = bcsp.tile([128, 512], F32, name="rc", tag="rc")
                    nc.vector.reciprocal_approx_fast(rc[:, :], dps[:, :])
                    ot = outp.tile([128, 512], F32, name="ot", tag="ot")
                    nc.vector.tensor_mul(ot[:, :], pav[0][:, :], rc[:, :])
                    ot2 = outp.tile([128, 512], F32, name="ot2", tag="ot2")
                    nc.vector.tensor_scalar_add(ot2[:, :], ot[:, :],
                                                bvq_s[:, 0:1])
                    nc.sync.dma_start(
                        out=out[b_, :, qt_ * 512:(qt_ + 1) * 512],
                        in_=ot2[:, :])

                # batch-0 projection: first token-tile up front (enough for
                # wave 0's q and the first 4 k-tiles), rest via the filler
                segs0 = proj_groups(0)
                for seg in segs0[:3]:            # load0, qk(0,0), qk(0,1)
                    emit_group(seg)
                rest0 = segs0[3:]
                # reorder: remaining loads+qk first (k-data feeds the score
                # stream), v segments after
                filler.extend([s for s in rest0 if s[0] != "v"])
                filler.extend([s for s in rest0 if s[0] == "v"])

                prev = None        # (b, qt, pairs, esum) of previous wave
                pav = [None]
                for b in range(B):
                    for qt in range(QT):
                        if qt == 0 and b + 1 < B:
                            filler.extend(proj_groups(b + 1))
                        qcol = qt * 512
                        if prev is not None:
                            pav[0] = avps.tile([128, 512], F32, name="av",
                                               tag="av", bufs=1)
                        pairs = []
                        lvlA = []
                        lvlB = []
                        budget_per_kt = 5.2 if b == 0 else 2.6
                        for kt in range(KTOK_B):
                            kcol = kt * 128
                            half = kt % 2
                            if half == 0:
                                ep = expp.tile([128, 2048], BF16, name="e2",
                                               tag="e2")
                                pairs.append(ep)
                            s2 = sps.tile([128, 1024], F32, name="s2", tag="s2")
                            for h in range(2):
                                nc.tensor.matmul(
                                    s2[:, h * 512:(h + 1) * 512],
                                    lhsT=k_sb[b][h * 64:(h + 1) * 64, kcol:kcol + 128],
                                    rhs=q_sb[b][h * 64:(h + 1) * 64, qcol:qcol + 512],
                                    start=True, stop=True,
                                    tile_position=(h * 64, 0))
                            dst = pairs[-1][:, half * 1024:(half + 1) * 1024]
                            if kt in SCHRAUD_KT:
                                nc.vector.tensor_scalar(
                                    dst.bitcast(mybir.dt.int16), s2[:, :],
                                    SCH_A, SCH_B,
                                    mybir.AluOpType.mult, mybir.AluOpType.add)
                            else:
                                nc.scalar.activation(dst, s2[:, :], AF.Exp)
                            if kt % 4 == 3:
                                # previous wave's AV in runs of 8 MMs (fewer
                                # PE weight-switch penalties)
                                if prev is not None:
                                    for k2 in range(kt - 3, kt + 1):
                                        emit_av(prev[0], prev[2], k2)
                                # denominator add-tree (bf16, pairwise)
                                a = treep.tile([128, 2048], BF16, name="tA",
                                               tag="tA")
                                nc.vector.tensor_add(a[:, :], pairs[-2][:, :],
                                                     pairs[-1][:, :])
                                lvlA.append(a)
                                if len(lvlA) % 2 == 0:
                                    bt = treep.tile([128, 2048], BF16,
                                                    name="tB", tag="tB")
                                    nc.vector.tensor_add(bt[:, :],
                                                         lvlA[-2][:, :],
                                                         lvlA[-1][:, :])
                                    lvlB.append(bt)
                            if kt % 4 == 2:
                                budget = budget_per_kt
                                while filler and budget > 0:
                                    seg = filler.popleft()
                                    budget -= GCOST[seg[0]]
                                    emit_group(seg)
                        ct = treep.tile([128, 2048], BF16, name="tC", tag="tC")
                        nc.vector.tensor_add(ct[:, :], lvlB[0][:, :],
                                             lvlB[1][:, :])
                        esum = esump.tile([128, 1024], BF16, name="es",
                                          tag="es")
                        nc.vector.tensor_add(esum[:, :], ct[:, 0:1024],
                                             ct[:, 1024:2048])
                        if prev is not None:
                            emit_tail(prev[0], prev[1], prev[3])
                        prev = (b, qt, pairs, esum)
                # drain: last wave's AV + tail
                while filler:
                    emit_group(filler.popleft())
                pav[0] = avps.tile([128, 512], F32, name="av", tag="av",
                                   bufs=1)
                for kt in range(KTOK_B):
                    emit_av(prev[0], prev[2], kt)
                emit_tail(prev[0], prev[1], prev[3])
    nc.compile()
    return nc


_GRAPH = None


def _get_graph():
    global _GRAPH
    if _GRAPH is None:
        _GRAPH = build_graph()
    return _GRAPH


def _make_in_maps(x, w_qkv, b_qkv):
    bf = ml_dtypes.bfloat16
    xt = np.ascontiguousarray(x.reshape(TOK, DIM).T).astype(bf)
    in_maps = []
    for c in range(NCORES):
        hA, hB = HPC * c, HPC * c + 1
        rq = [w_qkv[h * HD:(h + 1) * HD] * SCALE for h in (hA, hB)]
        rk = [w_qkv[DIM + h * HD: DIM + (h + 1) * HD] for h in (hA, hB)]
        rv = [w_qkv[2 * DIM + h * HD: 2 * DIM + (h + 1) * HD] for h in (hA, hB)]
        wqk_c = np.ascontiguousarray(np.concatenate(rq + rk, axis=0).T).astype(bf)
        wv_c = np.ascontiguousarray(np.concatenate(rv, axis=0).T).astype(bf)
        bq = [b_qkv[h * HD:(h + 1) * HD] * SCALE for h in (hA, hB)]
        bk = [b_qkv[DIM + h * HD: DIM + (h + 1) * HD] for h in (hA, hB)]
        bvc = [b_qkv[2 * DIM + h * HD: 2 * DIM + (h + 1) * HD] for h in (hA, hB)]
        bqk_c = np.concatenate(bq + bk).astype(np.float32).reshape(-1, 1)
        bvq_c = np.concatenate(bvc).astype(np.float32).reshape(-1, 1)
        in_maps.append({"xt": xt, "wqk": wqk_c, "wv": wv_c,
                        "bqk": np.ascontiguousarray(bqk_c),
                        "bvq": np.ascontiguousarray(bvq_c)})
    return in_maps


def _run(x, w_qkv, b_qkv, trace=False, tmpdir=None):
    nc = _get_graph()
    in_maps = _make_in_maps(np.asarray(x, dtype=np.float32),
                            np.asarray(w_qkv, dtype=np.float32),
                            np.asarray(b_qkv, dtype=np.float32))
    res = run_bass_kernel_spmd(nc, in_maps, core_ids=list(range(NCORES)),
                               trace=trace, tmpdir=tmpdir)
    full = np.empty((B, N, DIM), dtype=np.float32)
    for c in range(NCORES):
        oc = res.results[c]["out"]          # [B, 128, N]
        full[:, :, c * HPC * HD:(c + 1) * HPC * HD] = oc.transpose(0, 2, 1)
    return full, res


def kernel(x, w_qkv, b_qkv):
    full, _ = _run(x, w_qkv, b_qkv, trace=False)
    return full
